# revision 7
# baseline (speedup 1.0000x reference)
"""Trainium2 Bass kernel for nn_BilinearInteractionLayer.

Reference computation (B=2048, F=16 fields, E=256, P=120 field pairs):
    row, col = triu_indices(16, k=1)
    out[b, p, :] = (emb[b, row[p], :] @ W[p]) * emb[b, col[p], :] + bias[p]

Strategy: shard the PAIR dimension across the 8 NeuronCores so the
(120,256,256) weight is read exactly once in total. All cores run one SPMD
program; per-core pair assignments are expressed through a shared "local
graph" G (slot-pair arc list) plus per-core slot->field maps, baked in below
(found by offline search). Everything on-device runs in transposed layout
[embed, batch] so that:
  - the matmul stationary operand is W (natural layout),
  - p and q field activations are the same tensors (embT),
  - the bias is a per-partition constant, added after the q-multiply by the
    Scalar engine (activation Identity with a per-partition bias vector),
  - the epilogue is one DVE multiply + one in-place ACT bias-add per tile.
Matmuls run as float32r (one PE pass, ~13 mantissa bits).
Host packs inputs into device layouts and unpacks the transposed output.
"""

import numpy as np

import concourse.bacc as bacc
import concourse.tile as tile
import concourse.mybir as mybir
from concourse.bass_utils import run_bass_kernel_spmd

# ---------------------------------------------------------------- constants
B = 2048          # batch
E = 256           # embed
NF = 16           # fields
NCORES = 8
BCH = 512         # batch chunk on device
NCH = B // BCH    # 4 chunks
F32 = mybir.dt.float32
F32R = mybir.dt.float32r

# Pair-assignment design: shared directed arc list on SLOTS slots + per-core
# slot->field maps. (phi[c][u], phi[c][v]) for each arc (u, v) enumerates the
# global ordered pairs this core computes; kernel() consults PRIMARY to
# decide which core's copy of each global pair is used.
DESIGN = {
    "slots": 8,
    "arcs": [(0, 4), (0, 6), (0, 7), (2, 4), (2, 6), (0, 5), (1, 4), (3, 4),
             (5, 4), (0, 1), (0, 2), (0, 3), (2, 7), (3, 6), (6, 7), (1, 7),
             (2, 5), (3, 7)],
    "phis": [[2, 3, 4, 11, 13, 9, 12, 14],
             [4, 3, 2, 6, 8, 5, 10, 7],
             [1, 2, 7, 0, 15, 11, 8, 10],
             [0, 7, 1, 11, 4, 3, 5, 14],
             [3, 5, 9, 8, 11, 10, 14, 15],
             [5, 6, 7, 8, 9, 10, 12, 13],
             [0, 2, 1, 3, 12, 9, 13, 6],
             [10, 4, 6, 13, 15, 12, 14, 11]],
}

SLOTS = DESIGN["slots"]
ARCS = DESIGN["arcs"]
PHIS = DESIGN["phis"]
NA = len(ARCS)
GRP = 5  # pairs per output-DMA group

_ROW, _COL = np.triu_indices(NF, k=1)
_PAIR_ID = {}
for _k in range(len(_ROW)):
    _PAIR_ID[(int(_ROW[_k]), int(_COL[_k]))] = _k


def _core_tables():
    """Per-core: local arc k -> global pair id (or -1 garbage), and the
    global primary map pair_id -> (core, local_k)."""
    local_gp = np.full((NCORES, NA), -1, dtype=np.int64)
    primary = {}
    for c in range(NCORES):
        phi = PHIS[c]
        for k, (u, v) in enumerate(ARCS):
            a, b = phi[u], phi[v]
            if a < b:
                pid = _PAIR_ID[(a, b)]
                local_gp[c, k] = pid
                if pid not in primary:
                    primary[pid] = (c, k)
    return local_gp, primary


_LOCAL_GP, _PRIMARY = _core_tables()

# ---------------------------------------------------------------- program

_PROGRAM = None


def _emit_body(nc, tc, fields_v, wts_d, biasp_d, out_v):
    """Emit one full pass of the computation (all chunks, all pairs)."""
    with tc.tile_pool(name="wpool", bufs=1) as wpool, \
         tc.tile_pool(name="cpool", bufs=1) as cpool, \
         tc.tile_pool(name="fpool", bufs=2) as fpool, \
         tc.tile_pool(name="opool", bufs=3) as opool, \
         tc.tile_pool(name="psum", bufs=6, space="PSUM") as pspool:

        w_sb = wpool.tile([128, NA * 2 * 2 * 128], F32R)
        nc.sync.dma_start(w_sb[:], wts_d[:])
        w_v = w_sb[:].rearrange("p (k et ft f) -> p k et ft f",
                                k=NA, et=2, ft=2, f=128)

        bias_sb = cpool.tile([128, NA * 2], F32)
        nc.sync.dma_start(bias_sb[:], biasp_d[:])

        for ch in range(NCH):
            f_sb = fpool.tile([128, SLOTS * 2 * BCH], F32R, tag="fields")
            nc.sync.dma_start(
                f_sb[:], fields_v[:, :, :, ch * BCH:(ch + 1) * BCH])
            f_v = f_sb[:].rearrange("p (s et b) -> p s et b",
                                    s=SLOTS, et=2, b=BCH)

            ngrp = (NA + GRP - 1) // GRP
            for g in range(ngrp):
                ks = list(range(g * GRP, min((g + 1) * GRP, NA)))
                o_sb = opool.tile([128, len(ks) * 2 * BCH], F32,
                                  tag="ostage")
                o_v = o_sb[:].rearrange("p (k ft b) -> p k ft b",
                                        k=len(ks), ft=2, b=BCH)
                for kk, k in enumerate(ks):
                    u, v = ARCS[k]
                    for ft in range(2):
                        ps = pspool.tile([128, BCH], F32)
                        nc.tensor.matmul(
                            ps[:], w_v[:, k, 0, ft, :], f_v[:, u, 0, :],
                            start=True, stop=False)
                        nc.tensor.matmul(
                            ps[:], w_v[:, k, 1, ft, :], f_v[:, u, 1, :],
                            start=False, stop=True)
                        nc.vector.tensor_mul(
                            o_v[:, kk, ft, :], ps[:],
                            f_v[:, v, ft, :].bitcast(F32))
                        nc.scalar.activation(
                            o_v[:, kk, ft, :], o_v[:, kk, ft, :],
                            mybir.ActivationFunctionType.Identity,
                            bias=bias_sb[:, k * 2 + ft:k * 2 + ft + 1])
                nc.sync.dma_start(
                    out_v[:, ks[0]:ks[-1] + 1, :, ch * BCH:(ch + 1) * BCH],
                    o_v)


def _build_program(niter=None):
    """niter=None: the real kernel (external I/O). niter=N: a timing build
    whose body runs N times on internal (garbage) DRAM scratch."""
    nc = bacc.Bacc("TRN2", target_bir_lowering=False, debug=False,
                   num_devices=NCORES)
    kind_in = {} if niter else {"kind": "ExternalInput"}
    kind_out = {} if niter else {"kind": "ExternalOutput"}
    fields_d = nc.dram_tensor("fields", [128, SLOTS * 2 * B], F32R,
                              **kind_in).ap()
    wts_d = nc.dram_tensor("wts", [128, NA * 2 * 2 * 128], F32R,
                           **kind_in).ap()
    biasp_d = nc.dram_tensor("biasp", [128, NA * 2], F32, **kind_in).ap()
    out_d = nc.dram_tensor("out", [128, NA * 2 * B], F32, **kind_out).ap()

    fields_v = fields_d.rearrange("p (s et b) -> p s et b",
                                  s=SLOTS, et=2, b=B)
    out_v = out_d.rearrange("p (k ft b) -> p k ft b", k=NA, ft=2, b=B)

    if niter:
        tok_d = nc.dram_tensor("tok", [1, 8], F32, kind="ExternalOutput").ap()

    with tile.TileContext(nc) as tc:
        if niter:
            with tc.For_i(0, niter, 1):
                _emit_body(nc, tc, fields_v, wts_d, biasp_d, out_v)
            with tc.tile_pool(name="tokp", bufs=1) as tokp:
                tk = tokp.tile([1, 8], F32)
                nc.vector.memset(tk[:], 1.0)
                nc.sync.dma_start(tok_d[:], tk[:])
        else:
            _emit_body(nc, tc, fields_v, wts_d, biasp_d, out_v)

    nc.compile()
    return nc


def _get_program():
    global _PROGRAM
    if _PROGRAM is None:
        _PROGRAM = _build_program()
    return _PROGRAM


# ---------------------------------------------------------------- host side

def _pack_inputs(emb, weight, bias):
    """Build the 8 per-core input maps."""
    # emb (B, NF, E) -> embT (2, 128, NF, B): embT[et, e_lo, f, b]
    embT = np.ascontiguousarray(emb.transpose(2, 1, 0)).reshape(2, 128, NF, B)
    in_maps = []
    for c in range(NCORES):
        phi = list(PHIS[c])
        # fields[e_lo, s, et, b]
        fc = np.ascontiguousarray(
            embT[:, :, phi, :].transpose(1, 2, 0, 3)).reshape(
                128, SLOTS * 2 * B)
        gps = _LOCAL_GP[c]
        safe = np.where(gps >= 0, gps, 0)
        w = weight[safe]                      # (NA, 256, 256)
        wts = np.ascontiguousarray(
            w.reshape(NA, 2, 128, 2, 128).transpose(2, 0, 1, 3, 4)).reshape(
                128, NA * 2 * 2 * 128)
        bp = np.ascontiguousarray(
            bias[safe].reshape(NA, 2, 128).transpose(2, 0, 1)).reshape(
                128, NA * 2)
        in_maps.append({"fields": fc, "wts": wts, "biasp": bp})
    return in_maps


def _unpack_outputs(results):
    out = np.empty((B, len(_ROW), E), dtype=np.float32)
    per_core = {}
    for pid, (c, k) in _PRIMARY.items():
        if c not in per_core:
            # out_dev [128, NA, 2, B] -> [B, NA, 2*128=E]
            per_core[c] = results[c]["out"].reshape(
                128, NA, 2, B).transpose(3, 1, 2, 0).reshape(B, NA, E)
        out[:, pid, :] = per_core[c][:, k, :]
    return out


def kernel(emb_inputs, weight, bias):
    assert len(_PRIMARY) == len(_ROW), (
        f"design covers {len(_PRIMARY)}/{len(_ROW)} pairs")
    nc = _get_program()
    in_maps = _pack_inputs(np.asarray(emb_inputs), np.asarray(weight),
                           np.asarray(bias))
    res = run_bass_kernel_spmd(nc, in_maps, list(range(NCORES)))
    return _unpack_outputs(res.results)
